# revision 14
# baseline (speedup 1.0000x reference)
"""Trainium2 Bass kernel for nn_DecoderTransformer segment_reduce problem.

Computes, per batch sample b (one NeuronCore each, 8 cores total):
    sums[s, :]   = sum over (n, k) with indexes[b, n, k] == s of graph_output[b, n, :]
    counts[s]    = multiplicity of s in indexes[b]
    graph_hidden = (sums + 1e-8) / max(counts, 1)
    enc[b]       = concat([graph_hidden, seq_output[b]], axis=-1)   # [2048, 1024]
Returns (enc [8, 2048, 1024] f32, hidden [8, 1024] f32 passthrough).

Device algorithm (per core):
  Host sorts the 2048 (n, k) updates by target s; the sorted stream is cut in
  16 chunks of 128. graph_output is host-split into an exact bf16 hi/lo pair
  packed per row (row n = [bf16(G[n]), bf16(G[n] - bf16(G[n]))], 2 KiB), so a
  single indirect-DMA row gather per chunk delivers both matmul operands with
  no on-chip splitting. Per chunk, one is_equal tensor_scalar against a
  host-provided iota row builds the bf16 selection matrix for the window of
  output tiles the chunk's targets span (window union across cores keeps the
  program SPMD-uniform). The scatter-add is Sel.T @ rows on the tensor engine
  as two bf16 matmuls (hi+lo, shared stationary operand), accumulated in fp32
  PSUM across chunks; the hi/lo split keeps ~17 mantissa bits (~1e-5 worst
  rel err vs fp32). The PSUM->SBUF pass fuses (sums + eps) * r with
  r = 1/max(counts, 1) host-precomputed from the index metadata, writing the
  left half of [128, 4096] super-tiles whose right half receives seq_output;
  each 128-row tile stores as one contiguous 512 KiB DMA as soon as it
  finishes. Metadata rides the otherwise-idle SWDGE pool ring so its
  completion is not starved by the seq_output prefetch flood. Instructions
  are emitted chunk-major in execution order (per-engine queues are FIFO).
"""

import numpy as np
import ml_dtypes

import concourse.bass as bass
import concourse.bacc as bacc
import concourse.tile as tile
from concourse import mybir
from concourse.bass_utils import run_bass_kernel_spmd

B, S, N, K = 8, 2048, 512, 4
DG, DSEQ, H = 512, 512, 1024
P = 128
N_CHUNKS = (N * K) // P  # 16
N_TILES = S // P  # 16
TPS = 4  # tiles per output super-tile (seq staging granularity)
EPS = 1e-8

# Filled by kernel() on every call; read by test harnesses.
LAST_EXEC_NS = None
LAST_RESULTS = None


def _host_metadata(indexes):
    """Per-core sorted-update metadata + SPMD-uniform chunk->tile windows."""
    per_core = []
    for b in range(B):
        t_flat = np.asarray(indexes[b], dtype=np.int64).reshape(-1)  # (n, k) order
        order = np.argsort(t_flat, kind="stable")
        t_sorted = t_flat[order]
        src = (order // K).astype(np.int32)
        counts = np.bincount(t_flat, minlength=S)
        r = (1.0 / np.maximum(counts, 1)).astype(np.float32)
        per_core.append((t_sorted, src, r))

    # Union coverage: chunk c touches output tiles [lo[c], hi[c]] across cores.
    lo = np.full(N_CHUNKS, N_TILES, np.int64)
    hi = np.full(N_CHUNKS, -1, np.int64)
    for t_sorted, _, _ in per_core:
        tc_lo = t_sorted.reshape(N_CHUNKS, P)[:, 0] // P
        tc_hi = t_sorted.reshape(N_CHUNKS, P)[:, -1] // P
        lo = np.minimum(lo, tc_lo)
        hi = np.maximum(hi, tc_hi)
    lo = lo.astype(int)
    hi = hi.astype(int)

    src_cols = np.zeros((B, P, N_CHUNKS), np.int32)  # [p, c] -> source row
    shift_cols = np.zeros((B, P, N_CHUNKS), np.float32)
    r_cols = np.zeros((B, P, N_TILES), np.float32)
    for b in range(B):
        t_sorted, src, r = per_core[b]
        src_cols[b] = src.reshape(N_CHUNKS, P).T.astype(np.int32)
        r_cols[b] = r.reshape(N_TILES, P).T
        ts_chunks = t_sorted.reshape(N_CHUNKS, P)
        for c in range(N_CHUNKS):
            shift_cols[b, :, c] = (ts_chunks[c] - P * lo[c]).astype(np.float32)
    return lo, hi, src_cols, shift_cols, r_cols


def _build_kernel(lo, hi):
    f32 = mybir.dt.float32
    bf16 = mybir.dt.bfloat16
    spans = [hi[c] - lo[c] + 1 for c in range(N_CHUNKS)]
    wmax = P * max(spans)
    # per tile: contributing chunks (ordered) for PSUM start/stop flags
    tile_chunks = {tau: [c for c in range(N_CHUNKS) if lo[c] <= tau <= hi[c]]
                   for tau in range(N_TILES)}

    nc = bacc.Bacc("TRN2", target_bir_lowering=False, debug=False)
    seq = nc.dram_tensor("seq", [S, DSEQ], f32, kind="ExternalInput")
    g2 = nc.dram_tensor("g2", [N, 2 * DG], bf16, kind="ExternalInput")
    srcm = nc.dram_tensor("srcm", [P, N_CHUNKS], mybir.dt.int32,
                          kind="ExternalInput")
    # shift | r | iota bundled into one f32 metadata tensor
    metam = nc.dram_tensor("metam", [P, N_CHUNKS + N_TILES + wmax], f32,
                           kind="ExternalInput")
    enc = nc.dram_tensor("enc", [S, DG + DSEQ], f32, kind="ExternalOutput")

    with tile.TileContext(nc) as tc:
        with (
            tc.tile_pool(name="const", bufs=1) as const,
            tc.tile_pool(name="gath", bufs=N_CHUNKS) as gather_pool,
            tc.tile_pool(name="eq", bufs=N_CHUNKS) as eq_pool,
            tc.tile_pool(name="out", bufs=4) as out_pool,
            tc.tile_pool(name="psum", bufs=8, space="PSUM") as psum_pool,
        ):
            # --- Pool(SWDGE) ring: metadata first (this ring is otherwise
            # idle until the gathers; the HWDGE rings get flooded by the seq
            # prefetch, which starves small-DMA completions by ~8 us).
            src_sb = const.tile([P, N_CHUNKS], mybir.dt.int32)
            nc.gpsimd.dma_start(out=src_sb[:], in_=srcm[:, :])
            meta_sb = const.tile([P, N_CHUNKS + N_TILES + wmax], f32)
            nc.gpsimd.dma_start(out=meta_sb[:], in_=metam[:, :])
            shift_sb = meta_sb[:, 0:N_CHUNKS]
            r_sb = meta_sb[:, N_CHUNKS:N_CHUNKS + N_TILES]
            iota_f = meta_sb[:, N_CHUNKS + N_TILES:]

            # --- Pool ring: 16 per-chunk row gathers (single-offset indirect
            # DMA; each gathered row carries the packed bf16 hi|lo pair).
            gathers = []
            for c in range(N_CHUNKS):
                gt = gather_pool.tile([P, 2 * DG], bf16, name=f"gt{c}", tag="gt")
                nc.gpsimd.indirect_dma_start(
                    out=gt[:], out_offset=None, in_=g2[:, :],
                    in_offset=bass.IndirectOffsetOnAxis(
                        ap=src_sb[:, c:c + 1], axis=0),
                )
                gathers.append(gt)

            # --- Scalar(ACT) HWDGE ring: seq rows into the out super-tiles.
            ots = []
            for st in range(N_TILES // TPS):
                ot = out_pool.tile([P, TPS, DG + DSEQ], f32)
                nc.scalar.dma_start(
                    out=ot[:, :, DG:],
                    in_=seq[:, :].rearrange("(t p) d -> p t d", p=P)[
                        :, st * TPS:(st + 1) * TPS, :],
                )
                ots.append(ot)

            # epsr on DVE (needed late, by epilogues only)
            epsr_sb = const.tile([P, N_TILES], f32)
            nc.vector.tensor_scalar(
                out=epsr_sb[:], in0=r_sb, scalar1=EPS, scalar2=None,
                op0=mybir.AluOpType.mult,
            )

            # --- Chunk-major pipeline, emission order == execution order.
            psums = {}
            seen = {tau: 0 for tau in range(N_TILES)}

            def finish_tile(tau):
                """Fused epilogue (alternating DVE/ACT) + per-tile store."""
                st, ti = divmod(tau, TPS)
                oslice = ots[st][:, ti, :DG]
                if tau in psums:
                    if tau % 2 == 0:
                        nc.vector.tensor_scalar(
                            out=oslice, in0=psums[tau][:], scalar1=EPS,
                            scalar2=r_sb[:, tau:tau + 1],
                            op0=mybir.AluOpType.add, op1=mybir.AluOpType.mult,
                        )
                    else:
                        nc.scalar.activation(
                            out=oslice, in_=psums[tau][:],
                            func=mybir.ActivationFunctionType.Identity,
                            bias=epsr_sb[:, tau:tau + 1],
                            scale=r_sb[:, tau:tau + 1],
                        )
                else:
                    nc.vector.memset(oslice, EPS)
                nc.sync.dma_start(
                    out=enc[tau * P:(tau + 1) * P, :],
                    in_=ots[st][:, ti, :],
                )

            for c in range(N_CHUNKS):
                ghi = gathers[c][:, :DG]
                glo = gathers[c][:, DG:]
                # bf16 selection window (DVE; ready as soon as metadata lands)
                eqb = eq_pool.tile([P, P * spans[c]], bf16, tag="eqb")
                nc.vector.tensor_scalar(
                    out=eqb[:], in0=iota_f[:, :P * spans[c]],
                    scalar1=shift_sb[:, c:c + 1], scalar2=None,
                    op0=mybir.AluOpType.is_equal,
                )
                for tau in range(lo[c], hi[c] + 1):
                    clist = tile_chunks[tau]
                    if tau not in psums:
                        psums[tau] = psum_pool.tile([P, DG], f32, space="PSUM",
                                                    name=f"ps{tau}", tag="ps")
                    first = clist[0] == c
                    last = clist[-1] == c
                    off = P * (tau - lo[c])
                    nc.tensor.matmul(
                        out=psums[tau][:], lhsT=eqb[:, off:off + P], rhs=ghi,
                        start=first, stop=False,
                    )
                    nc.tensor.matmul(
                        out=psums[tau][:], lhsT=eqb[:, off:off + P], rhs=glo,
                        start=False, stop=last,
                    )
                    seen[tau] += 1
                    if seen[tau] == len(clist):
                        finish_tile(tau)
            # tiles covered by no chunk window (possible for adversarial index
            # distributions): plain eps fill
            for tau in range(N_TILES):
                if not tile_chunks[tau]:
                    finish_tile(tau)
    nc.compile()
    return nc


def kernel(seq_output, graph_output, hidden, indexes, _trace=False):
    global LAST_EXEC_NS, LAST_RESULTS
    seq_output = np.ascontiguousarray(np.asarray(seq_output, dtype=np.float32))
    graph_output = np.asarray(graph_output, dtype=np.float32)
    hidden_np = np.asarray(hidden)

    lo, hi, src_cols, shift_cols, r_cols = _host_metadata(indexes)
    nc = _build_kernel(lo, hi)

    wmax = P * max(hi[c] - lo[c] + 1 for c in range(N_CHUNKS))
    iota_row = np.arange(wmax, dtype=np.float32)

    # exact bf16 hi/lo split of graph_output, packed per row: [hi | lo]
    ghi = graph_output.astype(ml_dtypes.bfloat16)
    glo = (graph_output - ghi.astype(np.float32)).astype(ml_dtypes.bfloat16)
    g2 = np.concatenate([ghi, glo], axis=-1)  # [B, N, 1024] bf16

    in_maps = []
    for b in range(B):
        meta = np.concatenate(
            [shift_cols[b], r_cols[b],
             np.broadcast_to(iota_row, (P, wmax))], axis=1,
        ).astype(np.float32)
        in_maps.append({
            "seq": seq_output[b],
            "g2": np.ascontiguousarray(g2[b]),
            "srcm": np.ascontiguousarray(src_cols[b]),
            "metam": np.ascontiguousarray(meta),
        })
    res = run_bass_kernel_spmd(nc, in_maps, core_ids=list(range(B)), trace=_trace)
    LAST_EXEC_NS = res.exec_time_ns
    LAST_RESULTS = res
    enc = np.stack([res.results[b]["enc"] for b in range(B)], axis=0)
    hidden_flat = np.ascontiguousarray(hidden_np.reshape(hidden_np.shape[0], -1))
    return enc, hidden_flat


# revision 15
# speedup vs baseline: 1.3619x; 1.3619x over previous
"""Trainium2 Bass kernel for nn_DecoderTransformer segment_reduce problem.

Computes, per batch sample b (one NeuronCore each, 8 cores total):
    sums[s, :]   = sum over (n, k) with indexes[b, n, k] == s of graph_output[b, n, :]
    counts[s]    = multiplicity of s in indexes[b]
    graph_hidden = (sums + 1e-8) / max(counts, 1)
    enc[b]       = concat([graph_hidden, seq_output[b]], axis=-1)   # [2048, 1024]
Returns (enc [8, 2048, 1024] f32, hidden [8, 1024] f32 passthrough).

Device algorithm (per core):
  Host sorts the 2048 (n, k) updates by target s; the sorted stream is cut in
  16 chunks of 128. graph_output is host-split into an exact bf16 hi/lo pair
  packed per row (row n = [bf16(G[n]), bf16(G[n] - bf16(G[n]))], 2 KiB), so a
  single indirect-DMA row gather per chunk delivers both matmul operands with
  no on-chip splitting. Per chunk, one is_equal tensor_scalar against a
  host-provided iota row builds the bf16 selection matrix for the window of
  output tiles the chunk's targets span (window union across cores keeps the
  program SPMD-uniform). The scatter-add is Sel.T @ rows on the tensor engine
  as two bf16 matmuls (hi+lo, shared stationary operand), accumulated in fp32
  PSUM across chunks; the hi/lo split keeps ~17 mantissa bits (~1e-5 worst
  rel err vs fp32). The PSUM->SBUF pass fuses (sums + eps) * r with
  r = 1/max(counts, 1) host-precomputed from the index metadata, writing the
  left half of [128, 4096] super-tiles whose right half receives seq_output;
  each 128-row tile stores as one contiguous 512 KiB DMA as soon as it
  finishes. Metadata rides the otherwise-idle SWDGE pool ring so its
  completion is not starved by the seq_output prefetch flood. Instructions
  are emitted chunk-major in execution order (per-engine queues are FIFO).
"""

import numpy as np
import ml_dtypes

import concourse.bass as bass
import concourse.bacc as bacc
import concourse.tile as tile
from concourse import mybir
from concourse.bass_utils import run_bass_kernel_spmd

B, S, N, K = 8, 2048, 512, 4
DG, DSEQ, H = 512, 512, 1024
P = 128
N_CHUNKS = (N * K) // P  # 16
N_TILES = S // P  # 16
TPS = 4  # tiles per output super-tile (seq staging granularity)
EPS = 1e-8

# Filled by kernel() on every call; read by test harnesses.
LAST_EXEC_NS = None
LAST_RESULTS = None


def _host_metadata(indexes):
    """Per-core sorted-update metadata + SPMD-uniform chunk->tile windows."""
    per_core = []
    for b in range(B):
        t_flat = np.asarray(indexes[b], dtype=np.int64).reshape(-1)  # (n, k) order
        order = np.argsort(t_flat, kind="stable")
        t_sorted = t_flat[order]
        src = (order // K).astype(np.int32)
        counts = np.bincount(t_flat, minlength=S)
        r = (1.0 / np.maximum(counts, 1)).astype(np.float32)
        per_core.append((t_sorted, src, r))

    # Union coverage: chunk c touches output tiles [lo[c], hi[c]] across cores.
    lo = np.full(N_CHUNKS, N_TILES, np.int64)
    hi = np.full(N_CHUNKS, -1, np.int64)
    for t_sorted, _, _ in per_core:
        tc_lo = t_sorted.reshape(N_CHUNKS, P)[:, 0] // P
        tc_hi = t_sorted.reshape(N_CHUNKS, P)[:, -1] // P
        lo = np.minimum(lo, tc_lo)
        hi = np.maximum(hi, tc_hi)
    lo = lo.astype(int)
    hi = hi.astype(int)

    src_cols = np.zeros((B, P, N_CHUNKS), np.int32)  # [p, c] -> source row
    shift_cols = np.zeros((B, P, N_CHUNKS), np.float32)
    r_cols = np.zeros((B, P, N_TILES), np.float32)
    for b in range(B):
        t_sorted, src, r = per_core[b]
        src_cols[b] = src.reshape(N_CHUNKS, P).T.astype(np.int32)
        r_cols[b] = r.reshape(N_TILES, P).T
        ts_chunks = t_sorted.reshape(N_CHUNKS, P)
        for c in range(N_CHUNKS):
            shift_cols[b, :, c] = (ts_chunks[c] - P * lo[c]).astype(np.float32)
    return lo, hi, src_cols, shift_cols, r_cols


def _build_kernel(lo, hi):
    f32 = mybir.dt.float32
    bf16 = mybir.dt.bfloat16
    spans = [hi[c] - lo[c] + 1 for c in range(N_CHUNKS)]
    wmax = P * max(spans)
    # per tile: contributing chunks (ordered) for PSUM start/stop flags
    tile_chunks = {tau: [c for c in range(N_CHUNKS) if lo[c] <= tau <= hi[c]]
                   for tau in range(N_TILES)}

    nc = bacc.Bacc("TRN2", target_bir_lowering=False, debug=False)
    seq = nc.dram_tensor("seq", [S, DSEQ], f32, kind="ExternalInput")
    g2 = nc.dram_tensor("g2", [N, 2 * DG], bf16, kind="ExternalInput")
    srcm = nc.dram_tensor("srcm", [P, N_CHUNKS], mybir.dt.int32,
                          kind="ExternalInput")
    # shift | r | iota bundled into one f32 metadata tensor
    metam = nc.dram_tensor("metam", [P, N_CHUNKS + N_TILES + wmax], f32,
                           kind="ExternalInput")
    enc = nc.dram_tensor("enc", [S, DG + DSEQ], f32, kind="ExternalOutput")

    with tile.TileContext(nc) as tc:
        with (
            tc.tile_pool(name="const", bufs=1) as const,
            tc.tile_pool(name="gath", bufs=N_CHUNKS) as gather_pool,
            tc.tile_pool(name="eq", bufs=N_CHUNKS) as eq_pool,
            tc.tile_pool(name="out", bufs=4) as out_pool,
            tc.tile_pool(name="psum", bufs=8, space="PSUM") as psum_pool,
        ):
            # --- Sync HWDGE ring: gather offsets first (the first gather
            # keys off this completion), then the bundled metadata.
            src_sb = const.tile([P, N_CHUNKS], mybir.dt.int32)
            nc.sync.dma_start(out=src_sb[:], in_=srcm[:, :])
            meta_sb = const.tile([P, N_CHUNKS + N_TILES + wmax], f32)
            nc.sync.dma_start(out=meta_sb[:], in_=metam[:, :])
            shift_sb = meta_sb[:, 0:N_CHUNKS]
            r_sb = meta_sb[:, N_CHUNKS:N_CHUNKS + N_TILES]
            iota_f = meta_sb[:, N_CHUNKS + N_TILES:]

            # --- Pool ring: 16 per-chunk row gathers (single-offset indirect
            # DMA; each gathered row carries the packed bf16 hi|lo pair).
            gathers = []
            for c in range(N_CHUNKS):
                gt = gather_pool.tile([P, 2 * DG], bf16, name=f"gt{c}", tag="gt")
                nc.gpsimd.indirect_dma_start(
                    out=gt[:], out_offset=None, in_=g2[:, :],
                    in_offset=bass.IndirectOffsetOnAxis(
                        ap=src_sb[:, c:c + 1], axis=0),
                )
                gathers.append(gt)

            # --- Scalar(ACT) HWDGE ring: seq -> enc right half straight
            # DRAM->DRAM (no dependencies, no SBUF bounce); the dependent
            # output stream is then only the 4 MiB graph-hidden half.
            for q in range(4):
                nc.scalar.dma_start(
                    out=enc[q * S // 4:(q + 1) * S // 4, DG:],
                    in_=seq[q * S // 4:(q + 1) * S // 4, :],
                )
            ots = [out_pool.tile([P, TPS, DG], f32, name=f"ot{st}", tag="ot")
                   for st in range(N_TILES // TPS)]

            # epsr on DVE (needed late, by epilogues only)
            epsr_sb = const.tile([P, N_TILES], f32)
            nc.vector.tensor_scalar(
                out=epsr_sb[:], in0=r_sb, scalar1=EPS, scalar2=None,
                op0=mybir.AluOpType.mult,
            )

            # --- Chunk-major pipeline, emission order == execution order.
            psums = {}
            seen = {tau: 0 for tau in range(N_TILES)}

            def finish_tile(tau):
                """Fused epilogue (alternating DVE/ACT) + per-tile store."""
                st, ti = divmod(tau, TPS)
                oslice = ots[st][:, ti, :]
                if tau in psums:
                    if tau % 2 == 0:
                        nc.vector.tensor_scalar(
                            out=oslice, in0=psums[tau][:], scalar1=EPS,
                            scalar2=r_sb[:, tau:tau + 1],
                            op0=mybir.AluOpType.add, op1=mybir.AluOpType.mult,
                        )
                    else:
                        nc.scalar.activation(
                            out=oslice, in_=psums[tau][:],
                            func=mybir.ActivationFunctionType.Identity,
                            bias=epsr_sb[:, tau:tau + 1],
                            scale=r_sb[:, tau:tau + 1],
                        )
                else:
                    nc.vector.memset(oslice, EPS)
                nc.sync.dma_start(
                    out=enc[tau * P:(tau + 1) * P, :DG],
                    in_=ots[st][:, ti, :],
                )

            for c in range(N_CHUNKS):
                ghi = gathers[c][:, :DG]
                glo = gathers[c][:, DG:]
                # bf16 selection window (DVE; ready as soon as metadata lands)
                eqb = eq_pool.tile([P, P * spans[c]], bf16, tag="eqb")
                nc.vector.tensor_scalar(
                    out=eqb[:], in0=iota_f[:, :P * spans[c]],
                    scalar1=shift_sb[:, c:c + 1], scalar2=None,
                    op0=mybir.AluOpType.is_equal,
                )
                for tau in range(lo[c], hi[c] + 1):
                    clist = tile_chunks[tau]
                    if tau not in psums:
                        psums[tau] = psum_pool.tile([P, DG], f32, space="PSUM",
                                                    name=f"ps{tau}", tag="ps")
                    first = clist[0] == c
                    last = clist[-1] == c
                    off = P * (tau - lo[c])
                    nc.tensor.matmul(
                        out=psums[tau][:], lhsT=eqb[:, off:off + P], rhs=ghi,
                        start=first, stop=False,
                    )
                    nc.tensor.matmul(
                        out=psums[tau][:], lhsT=eqb[:, off:off + P], rhs=glo,
                        start=False, stop=last,
                    )
                    seen[tau] += 1
                    if seen[tau] == len(clist):
                        finish_tile(tau)
            # tiles covered by no chunk window (possible for adversarial index
            # distributions): plain eps fill
            for tau in range(N_TILES):
                if not tile_chunks[tau]:
                    finish_tile(tau)
    nc.compile()
    return nc


def kernel(seq_output, graph_output, hidden, indexes, _trace=False):
    global LAST_EXEC_NS, LAST_RESULTS
    seq_output = np.ascontiguousarray(np.asarray(seq_output, dtype=np.float32))
    graph_output = np.asarray(graph_output, dtype=np.float32)
    hidden_np = np.asarray(hidden)

    lo, hi, src_cols, shift_cols, r_cols = _host_metadata(indexes)
    nc = _build_kernel(lo, hi)

    wmax = P * max(hi[c] - lo[c] + 1 for c in range(N_CHUNKS))
    iota_row = np.arange(wmax, dtype=np.float32)

    # exact bf16 hi/lo split of graph_output, packed per row: [hi | lo]
    ghi = graph_output.astype(ml_dtypes.bfloat16)
    glo = (graph_output - ghi.astype(np.float32)).astype(ml_dtypes.bfloat16)
    g2 = np.concatenate([ghi, glo], axis=-1)  # [B, N, 1024] bf16

    in_maps = []
    for b in range(B):
        meta = np.concatenate(
            [shift_cols[b], r_cols[b],
             np.broadcast_to(iota_row, (P, wmax))], axis=1,
        ).astype(np.float32)
        in_maps.append({
            "seq": seq_output[b],
            "g2": np.ascontiguousarray(g2[b]),
            "srcm": np.ascontiguousarray(src_cols[b]),
            "metam": np.ascontiguousarray(meta),
        })
    res = run_bass_kernel_spmd(nc, in_maps, core_ids=list(range(B)), trace=_trace)
    LAST_EXEC_NS = res.exec_time_ns
    LAST_RESULTS = res
    enc = np.stack([res.results[b]["enc"] for b in range(B)], axis=0)
    hidden_flat = np.ascontiguousarray(hidden_np.reshape(hidden_np.shape[0], -1))
    return enc, hidden_flat
